# revision 13
# baseline (speedup 1.0000x reference)
"""Trainium2 Bass kernel for nn_CLIP_3v3d_brats (dense_cnn head + gated 1x1 conv).

v4 design:
  * NO collectives: 8 cores = batch(2) x 4 D-slabs of `pred`; each core
    computes the FULL GAP/MLP head for its own batch from a channel-major
    x_e slab [128ch, 13824pos] in bf16 (no cc barrier/AllReduce, no skew).
  * Head: GroupNorm stats chunked and overlapped with the slab DMA
    (ACT sumsq / DVE sum); prescaled R' = max(x+u,0) in ONE DVE
    tensor_scalar op (4x perf mode, ~0.3ns/col); 27 conv window sums
    split ACT(14, inline norm from slab)/DVE(13, reduce from R' with the
    GN scale folded into the W2 slice); x_feat accumulates via 27
    1-col-lhsT matmuls interleaved with window completion; tiny PE
    transposes to column orientation; gate MLP in bf16 (fast weight
    loads), 4x-replicated so the block-diag stream lhsT assembles with
    4 same-partition DVE copies.
  * Stream: bf16 pred; 108 chunk matmuls of [128,512] with outputs
    PARTITION-STACKED 3-deep per PSUM bank (tile_position col offsets
    0/32/64), so one [76,512] bias op drains 1536 output columns --
    drain cost drops 3x and the PE never starves (6 bank bufs).
    bf16 output DMA'd with a strided DRAM AP (18 stores).
"""
import sys
import types

sys.path.insert(0, "/opt/trn_rl_repo")

import numpy as np
import ml_dtypes

# Register the NTFF profile hook the agent image's antenv lacks (only
# needed when TRACE is enabled; harmless otherwise).
try:
    import antenv.axon_hooks  # noqa: F401
except ImportError:
    try:
        import trn_agent_boot.trn_boot as _tb

        _hooks = types.ModuleType("antenv.axon_hooks")
        _the_hook = _tb._ntff_profile_via_ctypes("/opt/axon/libaxon_pjrt.so")
        _hooks.get_axon_ntff_profile_hook = lambda: _the_hook
        _hooks.set_axon_ntff_profile_hook = lambda h: None
        sys.modules["antenv.axon_hooks"] = _hooks
    except Exception:
        pass

from concourse import bacc, tile, mybir
from concourse.bass_utils import run_bass_kernel_spmd

f32 = mybir.dt.float32
bf16 = mybir.dt.bfloat16
AF = mybir.ActivationFunctionType
ALU = mybir.AluOpType
AX = mybir.AxisListType
BF = ml_dtypes.bfloat16

N_CORES = 8
B = 2
K = 3
EPS = 1e-5
G = 4                      # position groups interleaved on partitions
NPOS = 221184              # positions per core slab: 24*96*96
NG = NPOS // G             # 55296
COLS = 6144                # stream tile columns (12 chunks of 512)
NITER = NG // COLS         # 9
NCH = COLS // 512          # 12 chunks per tile
NBANK = NCH // 3           # 4 banks per tile (3 stacks each)
NSLAB = 13824              # 24^3 positions of x_e per channel
NGRP = 8 * NSLAB           # elements per GN group (8 channels)
NSC = 6                    # slab DMA / stats chunks
CSC = NSLAB // NSC         # 2304
NWACT = 15                 # windows on ACT (inline); rest on DVE (from R')

TRACE = False
LAST_EXEC_NS = None
_CACHE = {}


def _build_program():
    nc = bacc.Bacc("TRN2", target_bir_lowering=False, debug=False,
                   num_devices=N_CORES)

    def din(name, shape, dt=f32):
        return nc.dram_tensor(name, shape, dt, kind="ExternalInput").ap()

    xe_slab_d = din("xe_slab", [128, NSLAB], bf16)
    pred_s = din("pred_s", [NITER, 128, COLS], bf16)
    w2dt_d = din("w2dt", [128, 27 * 256], bf16)
    gmask_d = din("gmask", [128, 16])
    gexp_d = din("gexp", [16, 128])
    gnw_d = din("gnw", [128, 2])
    gapbT_d = din("gapbT", [128, 2])
    ones13_d = din("ones13", [1, 3], bf16)
    gapbr_d = din("gapbr", [1, 256], bf16)
    bcrow_d = din("bcrow", [1, 256], bf16)
    ba1row_d = din("ba1row", [1, 16], bf16)
    ba2row_d = din("ba2row", [1, 128], bf16)
    w_cfT_d = din("w_cfT", [128, 2 * 512], bf16)
    bcf3_d = din("bcf3", [3, 4 * 128], bf16)
    id3_d = din("id3", [3, 3], bf16)
    w_cT_d = din("w_cT", [128, 4 * 256], bf16)
    w_a1T_d = din("w_a1T", [128, 2 * 16], bf16)
    w_a2T4_d = din("w_a2T4", [16, 128], bf16)
    wseg12_d = din("wseg12", [128, 3])
    bseg76_d = din("bseg76", [76, 1])

    out_d = nc.dram_tensor("out_s", [12, NG], bf16,
                           kind="ExternalOutput").ap()

    with tile.TileContext(nc) as tc:
        with tc.tile_pool(name="small", bufs=1) as sp, \
             tc.tile_pool(name="pred", bufs=8) as pp, \
             tc.tile_pool(name="outp", bufs=2) as op, \
             tc.tile_pool(name="hps", bufs=2, space="PSUM") as hps, \
             tc.tile_pool(name="sps", bufs=6, space="PSUM") as sps:
          with tc.tile_pool(name="headbig", bufs=1) as hb, \
               tc.tile_pool(name="scratch", bufs=2) as scp:

            # ---- head-critical loads first (sync-queue FIFO priority) ----
            xe_slab = hb.tile([128, NSLAB], bf16)
            for i in range(NSC):
                nc.sync.dma_start(xe_slab[:, i * CSC:(i + 1) * CSC],
                                  xe_slab_d[:, i * CSC:(i + 1) * CSC])
            gmask = sp.tile([128, 16], f32)
            nc.sync.dma_start(gmask[:], gmask_d[:])
            gexp = sp.tile([16, 128], f32)
            nc.sync.dma_start(gexp[:], gexp_d[:])
            gnw = sp.tile([128, 2], f32)
            nc.sync.dma_start(gnw[:], gnw_d[:])
            w2dt = hb.tile([128, 27 * 256], bf16)
            nc.sync.dma_start(w2dt[:], w2dt_d[:])
            gapbT = sp.tile([128, 2], f32)
            nc.sync.dma_start(gapbT[:], gapbT_d[:])
            ones13 = sp.tile([1, 3], bf16)
            nc.sync.dma_start(ones13[:], ones13_d[:])
            gapbr = sp.tile([1, 256], bf16)
            nc.sync.dma_start(gapbr[:], gapbr_d[:])
            bcrow = sp.tile([1, 256], bf16)
            nc.sync.dma_start(bcrow[:], bcrow_d[:])
            ba1row = sp.tile([1, 16], bf16)
            nc.sync.dma_start(ba1row[:], ba1row_d[:])
            ba2row = sp.tile([1, 128], bf16)
            nc.sync.dma_start(ba2row[:], ba2row_d[:])
            w_cfT = hb.tile([128, 2 * 512], bf16)
            nc.sync.dma_start(w_cfT[:], w_cfT_d[:])
            bcf3 = sp.tile([3, 4 * 128], bf16)
            nc.sync.dma_start(bcf3[:], bcf3_d[:])
            id3 = sp.tile([3, 3], bf16)
            nc.sync.dma_start(id3[:], id3_d[:])
            w_cT = hb.tile([128, 4 * 256], bf16)
            nc.sync.dma_start(w_cT[:], w_cT_d[:])
            w_a1T = sp.tile([128, 2 * 16], bf16)
            nc.sync.dma_start(w_a1T[:], w_a1T_d[:])
            w_a2T4 = sp.tile([16, 128], bf16)
            nc.sync.dma_start(w_a2T4[:], w_a2T4_d[:])
            wseg12 = sp.tile([128, 3], f32)
            nc.sync.dma_start(wseg12[:], wseg12_d[:])
            bd = sp.tile([128, 12], bf16)
            nc.vector.memset(bd[:], 0)
            bseg76 = sp.tile([76, 1], f32)
            nc.sync.dma_start(bseg76[:], bseg76_d[:])

            # ---- pred stream loads ----
            pts = []
            for t in range(NITER):
                pt = pp.tile([128, COLS], bf16, tag="pt")
                nc.sync.dma_start(pt[:], pred_s[t])
                pts.append(pt)

            # ---- GN stats, chunked to overlap the slab DMA ----
            sqp = sp.tile([128, NSC], f32)
            smp = sp.tile([128, NSC], f32)
            for i in range(NSC):
                ch_ = xe_slab[:, i * CSC:(i + 1) * CSC]
                sq_sc = scp.tile([128, CSC], bf16, tag="sc", bufs=1)
                nc.scalar.activation(sq_sc[:], ch_, AF.Square,
                                     accum_out=sqp[:, i:i + 1])
                nc.vector.tensor_reduce(smp[:, i:i + 1], ch_, AX.X, ALU.add)
            stat2 = sp.tile([128, 2], f32)  # cols: sum, sumsq
            nc.vector.tensor_reduce(stat2[:, 0:1], smp[:], AX.X, ALU.add)
            nc.vector.tensor_reduce(stat2[:, 1:2], sqp[:], AX.X, ALU.add)

            # group-sum via mask matmul -> [16, 2]
            g2 = hps.tile([16, 2], f32, tag="hps")
            nc.tensor.matmul(g2[:], gmask[:], stat2[:], start=True, stop=True)
            gsum = sp.tile([16, 2], f32)
            nc.vector.tensor_copy(gsum[:], g2[:])

            # -mu, rsqrt(var+eps) per group -> mr [16, 2]
            mr = sp.tile([16, 2], f32)  # cols: -mu, rs
            nc.scalar.mul(mr[:, 0:1], gsum[:, 0:1], -1.0 / NGRP)
            # n*var = sumsq + sum*(-mu); sd = sqrt(n*var/n + eps)
            nvar = sp.tile([16, 1], f32)
            nc.vector.scalar_tensor_tensor(nvar[:], gsum[:, 0:1],
                                           mr[:, 0:1], gsum[:, 1:2],
                                           ALU.mult, ALU.add)
            epst = sp.tile([16, 1], f32)
            nc.vector.memset(epst[:], float(EPS))
            sd = sp.tile([16, 1], f32)
            nc.scalar.activation(sd[:], nvar[:], AF.Sqrt,
                                 bias=epst[:, 0:1], scale=1.0 / NGRP)
            nc.vector.reciprocal(mr[:, 1:2], sd[:])

            # expand groups -> channels: chmr [128, 2] = (-mu_c, rs_c)
            ch2 = hps.tile([128, 2], f32, tag="hps")
            nc.tensor.matmul(ch2[:], gexp[:], mr[:], start=True, stop=True)
            chmr = sp.tile([128, 2], f32)
            nc.vector.tensor_copy(chmr[:], ch2[:])
            # scale_c = rs*gamma ; bias_c = beta + (-mu)*scale
            # u = bias/scale = beta/scale + (-mu)
            scale = sp.tile([128, 1], f32)
            nc.vector.tensor_scalar_mul(scale[:], chmr[:, 1:2], gnw[:, 0:1])
            bias = sp.tile([128, 1], f32)
            nc.vector.scalar_tensor_tensor(bias[:], chmr[:, 0:1], scale[:, 0:1],
                                           gnw[:, 1:2], ALU.mult, ALU.add)
            rsc = sp.tile([128, 1], f32)
            nc.vector.reciprocal(rsc[:], scale[:])
            u = sp.tile([128, 1], f32)
            nc.vector.scalar_tensor_tensor(u[:], gnw[:, 1:2], rsc[:, 0:1],
                                           chmr[:, 0:1], ALU.mult, ALU.add)

            # ---- R' = max(x+u, 0) in ONE DVE 4x op (assumes gamma>0) ----
            Rp = hb.tile([128, NSLAB], bf16)
            nc.vector.tensor_scalar(Rp[:], xe_slab[:], u[:, 0:1], 0.0,
                                    ALU.add, ALU.max)

            # ---- 27 window sums + interleaved x_feat accumulation ----
            S = sp.tile([128, 27], f32)
            Sb = sp.tile([128, 27], bf16)
            wsl = xe_slab[:].rearrange("p (d w h) -> p d w h",
                                       d=24, w=24, h=24)
            wrp = Rp[:].rearrange("p (d w h) -> p d w h", d=24, w=24, h=24)
            xfr = hps.tile([1, 256], f32, tag="hps")

            def xf_mm(o, first, last):
                nc.tensor.matmul(xfr[:], Sb[:, o:o + 1],
                                 w2dt[:, o * 256:(o + 1) * 256],
                                 start=first, stop=last)

            # DVE windows first (from R'), then ACT windows (inline norm)
            for o in range(NWACT, 27):
                od, ow, oh = o // 9, (o // 3) % 3, o % 3
                win = wrp[:, od:od + 21:2, ow:ow + 21:2, oh:oh + 21:2]
                nc.vector.tensor_reduce(S[:, o:o + 1], win, AX.XYZ, ALU.add)
                nc.vector.tensor_scalar(Sb[:, o:o + 1], S[:, o:o + 1],
                                        scale[:, 0:1], None, ALU.mult)
                xf_mm(o, o == NWACT, False)
            for o in range(NWACT):
                od, ow, oh = o // 9, (o // 3) % 3, o % 3
                win = wsl[:, od:od + 21:2, ow:ow + 21:2, oh:oh + 21:2]
                wscr = scp.tile([128, 1331], bf16, tag="wsc", bufs=1)
                nc.scalar.activation(wscr[:], win, AF.Relu,
                                     bias=bias[:, 0:1], scale=scale[:, 0:1],
                                     accum_out=S[:, o:o + 1])
                nc.vector.tensor_copy(Sb[:, o:o + 1], S[:, o:o + 1])
                xf_mm(o, False, o == NWACT - 1)

            xfsb = sp.tile([1, 256], bf16)
            nc.scalar.activation(xfsb[:], xfr[:], AF.Copy)
            # xcT chunks via broadcast matmuls: (x_feat + gap_b) x ones[1,3]
            xcT = sp.tile([128, 6], bf16)
            for pc in range(2):
                xcp = hps.tile([128, 3], f32, tag="hps")
                nc.tensor.matmul(xcp[:], xfsb[0:1, pc * 128:(pc + 1) * 128],
                                 ones13[:], start=True, stop=False)
                nc.tensor.matmul(xcp[:], gapbr[0:1, pc * 128:(pc + 1) * 128],
                                 ones13[:], start=False, stop=True)
                nc.vector.tensor_copy(xcT[:, pc * 3:pc * 3 + 3], xcp[:])

            # ---- MLP1: p3T = relu(Wx @ x_feat + (We@emb + b_cf)).T ----
            p3T = sp.tile([128, 4 * 3], bf16)
            p1 = hps.tile([128, 12], f32, tag="hps")
            for oc in range(4):
                for pc in range(2):
                    nc.tensor.matmul(
                        p1[:, oc * 3:oc * 3 + 3],
                        w_cfT[:, pc * 512 + oc * 128: pc * 512 + oc * 128 + 128],
                        xcT[:, pc * 3:pc * 3 + 3],
                        start=(pc == 0), stop=False)
                nc.tensor.matmul(p1[:, oc * 3:oc * 3 + 3],
                                 bcf3[:, oc * 128:(oc + 1) * 128],
                                 id3[:], start=False, stop=True)
            nc.scalar.activation(p3T[:], p1[:], AF.Relu)

            # ---- MLP2: c3T [128, 2*3] ----
            c3T = sp.tile([128, 2 * 3], bf16)
            c1 = hps.tile([128, 6], f32, tag="hps")
            for oc in range(2):
                for pc in range(4):
                    nc.tensor.matmul(
                        c1[:, oc * 3:oc * 3 + 3],
                        w_cT[:, pc * 256 + oc * 128: pc * 256 + oc * 128 + 128],
                        p3T[:, pc * 3:pc * 3 + 3],
                        start=(pc == 0), stop=False)
                nc.tensor.matmul(c1[:, oc * 3:oc * 3 + 3],
                                 bcrow[0:1, oc * 128:(oc + 1) * 128],
                                 ones13[:], start=False, stop=True)
            nc.scalar.activation(c3T[:], c1[:], AF.Identity)

            # ---- MLP3: hT [16, 3] ----
            h1 = hps.tile([16, 3], f32, tag="hps")
            for pc in range(2):
                nc.tensor.matmul(h1[:], w_a1T[:, pc * 16:pc * 16 + 16],
                                 c3T[:, pc * 3:pc * 3 + 3],
                                 start=(pc == 0), stop=False)
            nc.tensor.matmul(h1[:], ba1row[:], ones13[:],
                             start=False, stop=True)
            hT = sp.tile([16, 3], bf16)
            nc.scalar.activation(hT[:], h1[:], AF.Relu)

            # ---- MLP4, 4x-replicated: gT [128, 3] = sigmoid(...) ----
            g1 = hps.tile([128, 3], f32, tag="hps")
            nc.tensor.matmul(g1[:], w_a2T4[:], hT[:], start=True, stop=False)
            nc.tensor.matmul(g1[:], ba2row[:], ones13[:],
                             start=False, stop=True)
            gT = sp.tile([128, 3], f32)
            nc.scalar.activation(gT[:], g1[:], AF.Sigmoid)

            # ---- effw -> block-diagonal bf16 lhsT [128, 12], no DMA ----
            for g in range(G):
                nc.vector.tensor_mul(bd[32 * g:32 * g + 32, 3 * g:3 * g + 3],
                                     gT[32 * g:32 * g + 32, :],
                                     wseg12[32 * g:32 * g + 32, :])

          # ---- main stream: 3-stacked matmuls, wide drains ----
          # out_s col layout: ((t*3 + s)*NBANK + b)*512 + i  (host reorders)
          for t in range(NITER):
            so = op.tile([76, NBANK * 512], bf16, tag="so")
            for b in range(NBANK):
              po = sps.tile([128, 512], f32, tag="po")
              for s in range(3):
                j = b * 3 + s
                nc.tensor.matmul(po[32 * s:32 * s + 12, :], bd[:],
                                 pts[t][:, j * 512:(j + 1) * 512],
                                 start=True, stop=True)
              dst = so[:, b * 512:(b + 1) * 512]
              if b % 2 == 0:
                  nc.scalar.activation(dst, po[0:76, :], AF.Identity,
                                       bias=bseg76[:, 0:1])
              else:
                  nc.vector.tensor_scalar_add(dst, po[0:76, :],
                                              bseg76[:, 0:1])
            for s in range(3):
              c0 = (t * 3 + s) * NBANK * 512
              nc.gpsimd.dma_start(out_d[:, c0:c0 + NBANK * 512],
                                  so[32 * s:32 * s + 12, :])

    nc.compile()
    return nc


def _prep_shared(inp):
    """Host-side weight transposes shared by all cores."""
    gn_g = np.asarray(inp["gn_g"], np.float32)
    gn_b = np.asarray(inp["gn_b"], np.float32)
    gap_b = np.asarray(inp["gap_b"], np.float32)
    gap_w = np.asarray(inp["gap_w"], np.float32)
    w_cf = np.asarray(inp["w_cf"], np.float32)
    b_cf = np.asarray(inp["b_cf"], np.float32)
    w_c = np.asarray(inp["w_c"], np.float32)
    b_c = np.asarray(inp["b_c"], np.float32)
    w_a1 = np.asarray(inp["w_a1"], np.float32)
    b_a1 = np.asarray(inp["b_a1"], np.float32)
    w_a2 = np.asarray(inp["w_a2"], np.float32)
    b_a2 = np.asarray(inp["b_a2"], np.float32)
    emb = np.asarray(inp["emb"], np.float32)
    w_seg = np.asarray(inp["w_seg"], np.float32)
    b_seg = np.asarray(inp["b_seg"], np.float32)

    assert np.all(gn_g > 0), "kernel assumes positive GroupNorm gamma"

    p = np.arange(128)
    gmask = (p[:, None] // 8 == np.arange(16)[None, :]).astype(np.float32)
    gexp = np.ascontiguousarray(gmask.T)

    # W2 transposed + mean-fold: w2dt[c, o*256+oc] = gap_w[oc, c, o]/1331
    w2 = gap_w.reshape(256, 128, 27)
    w2dt = np.ascontiguousarray(
        (w2.transpose(1, 2, 0) / np.float32(1331.0)).reshape(128, 27 * 256)
    ).astype(BF)

    # x-half of w_cf, transposed: [128, 2*512]
    wx = w_cf[:, 0:256].T                            # [256, 512]
    w_cfT = np.concatenate(
        [wx[128 * pc:128 * (pc + 1), :] for pc in range(2)],
        axis=1).astype(BF)
    # constant-folded emb-half + bias: bcf3[k, o] = b_cf[o] + We @ emb_k
    bcf3 = np.ascontiguousarray(
        b_cf[None, :] + emb @ w_cf[:, 256:512].T).astype(BF)    # [3, 512]
    id3 = np.eye(3, dtype=np.float32).astype(BF)
    w_cT = np.concatenate(
        [w_c.T[128 * pc:128 * (pc + 1), :] for pc in range(4)],
        axis=1).astype(BF)
    bcT = np.ascontiguousarray(b_c.reshape(2, 128).T)
    w_a1T = np.concatenate(
        [w_a1.T[128 * pc:128 * (pc + 1), :] for pc in range(2)],
        axis=1).astype(BF)
    ba1 = b_a1.reshape(16, 1)
    w_a2T4 = np.ascontiguousarray(np.tile(w_a2.T, (1, 4))).astype(BF)
    ba2r = np.tile(b_a2, 4).reshape(128, 1)
    wseg12 = np.ascontiguousarray(np.tile(w_seg.T, (4, 1)))  # [128, 3]

    gapbT = np.ascontiguousarray(gap_b.reshape(2, 128).T)    # [128, 2]
    gnw = np.ascontiguousarray(np.stack([gn_g, gn_b], axis=1))  # [128, 2]
    ones13 = np.ones((1, 3), np.float32).astype(BF)
    gapbr = gap_b.reshape(1, 256).astype(BF)
    bcrow = b_c.reshape(1, 256).astype(BF)
    ba1row = b_a1.reshape(1, 16).astype(BF)
    ba2row = np.tile(b_a2, 4).reshape(1, 128).astype(BF)
    bseg76 = np.zeros((76, 1), np.float32)
    for s in range(3):
        bseg76[32 * s:32 * s + 12, 0] = np.tile(b_seg, 4)[:12]

    return dict(gmask=gmask, gexp=gexp, gnw=gnw, w2dt=w2dt, gapbT=gapbT,
                ones13=ones13, gapbr=gapbr, bcrow=bcrow, ba1row=ba1row,
                ba2row=ba2row, w_cfT=w_cfT, bcf3=bcf3, id3=id3, w_cT=w_cT,
                w_a1T=w_a1T, w_a2T4=w_a2T4,
                wseg12=wseg12, bseg76=bseg76)


def kernel(**inputs):
    global LAST_EXEC_NS
    x_e = np.asarray(inputs["x_e"], np.float32)
    pred = np.asarray(inputs["pred"], np.float32)

    shared = _prep_shared(inputs)
    shared = {k: (np.ascontiguousarray(v) if v.dtype == BF
                  else np.ascontiguousarray(v, dtype=np.float32))
              for k, v in shared.items()}

    xe_bf = x_e.reshape(B, 128, NSLAB).astype(BF)        # [2, 128, 13824]
    pred_bf = pred.astype(BF)                            # [2, 32, 96, 96, 96]

    in_maps = []
    for r in range(N_CORES):
        b, dq = divmod(r, 4)
        m = dict(shared)

        ps = pred_bf[b, :, dq * 24:(dq + 1) * 24]        # [32, 24, 96, 96]
        ps = ps.reshape(32, G, NITER, COLS).transpose(2, 1, 0, 3)
        m["pred_s"] = np.ascontiguousarray(ps.reshape(NITER, 128, COLS))
        m["xe_slab"] = np.ascontiguousarray(xe_bf[b])
        in_maps.append(m)

    if "nc" not in _CACHE:
        _CACHE["nc"] = _build_program()
    nc = _CACHE["nc"]

    res = run_bass_kernel_spmd(nc, in_maps, list(range(N_CORES)),
                               trace=TRACE)
    LAST_EXEC_NS = res.exec_time_ns

    out = np.empty((B, K, 96, 96, 96), np.float32)
    for r in range(N_CORES):
        b, dq = divmod(r, 4)
        o = np.asarray(res.results[r]["out_s"]).astype(np.float32)  # [12, NG]
        # stored col order ((t*3+s)*NBANK+b)*512+i -> logical chunk t*NCH+b*3+s
        o = o.reshape(12, NITER, 3, NBANK, 512).transpose(0, 1, 3, 2, 4)
        o = o.reshape(12, NG)
        o = o.reshape(G, K, NG).transpose(1, 0, 2).reshape(K, NPOS)
        out[b, :, dq * 24:(dq + 1) * 24] = o.reshape(K, 24, 96, 96)
    return out


# revision 14
# speedup vs baseline: 1.0119x; 1.0119x over previous
"""Trainium2 Bass kernel for nn_CLIP_3v3d_brats (dense_cnn head + gated 1x1 conv).

v4 design:
  * NO collectives: 8 cores = batch(2) x 4 D-slabs of `pred`; each core
    computes the FULL GAP/MLP head for its own batch from a channel-major
    x_e slab [128ch, 13824pos] in bf16 (no cc barrier/AllReduce, no skew).
  * Head: GroupNorm stats chunked and overlapped with the slab DMA
    (ACT sumsq / DVE sum); prescaled R' = max(x+u,0) in ONE DVE
    tensor_scalar op (4x perf mode, ~0.3ns/col); 27 conv window sums
    split ACT(14, inline norm from slab)/DVE(13, reduce from R' with the
    GN scale folded into the W2 slice); x_feat accumulates via 27
    1-col-lhsT matmuls interleaved with window completion; tiny PE
    transposes to column orientation; gate MLP in bf16 (fast weight
    loads), 4x-replicated so the block-diag stream lhsT assembles with
    4 same-partition DVE copies.
  * Stream: bf16 pred; 108 chunk matmuls of [128,512] with outputs
    PARTITION-STACKED 3-deep per PSUM bank (tile_position col offsets
    0/32/64), so one [76,512] bias op drains 1536 output columns --
    drain cost drops 3x and the PE never starves (6 bank bufs).
    bf16 output DMA'd with a strided DRAM AP (18 stores).
"""
import sys
import types

sys.path.insert(0, "/opt/trn_rl_repo")

import numpy as np
import ml_dtypes

# Register the NTFF profile hook the agent image's antenv lacks (only
# needed when TRACE is enabled; harmless otherwise).
try:
    import antenv.axon_hooks  # noqa: F401
except ImportError:
    try:
        import trn_agent_boot.trn_boot as _tb

        _hooks = types.ModuleType("antenv.axon_hooks")
        _the_hook = _tb._ntff_profile_via_ctypes("/opt/axon/libaxon_pjrt.so")
        _hooks.get_axon_ntff_profile_hook = lambda: _the_hook
        _hooks.set_axon_ntff_profile_hook = lambda h: None
        sys.modules["antenv.axon_hooks"] = _hooks
    except Exception:
        pass

from concourse import bacc, tile, mybir
from concourse.bass_utils import run_bass_kernel_spmd

f32 = mybir.dt.float32
bf16 = mybir.dt.bfloat16
AF = mybir.ActivationFunctionType
ALU = mybir.AluOpType
AX = mybir.AxisListType
BF = ml_dtypes.bfloat16

N_CORES = 8
B = 2
K = 3
EPS = 1e-5
G = 4                      # position groups interleaved on partitions
NPOS = 221184              # positions per core slab: 24*96*96
NG = NPOS // G             # 55296
COLS = 6144                # stream tile columns (12 chunks of 512)
NITER = NG // COLS         # 9
NCH = COLS // 512          # 12 chunks per tile
NBANK = NCH // 3           # 4 banks per tile (3 stacks each)
NSLAB = 13824              # 24^3 positions of x_e per channel
NGRP = 8 * NSLAB           # elements per GN group (8 channels)
NSC = 12                   # slab DMA / stats chunks
CSC = NSLAB // NSC         # 1152
NWACT = 15                 # windows on ACT (inline); rest on DVE (from R')

TRACE = False
LAST_EXEC_NS = None
_CACHE = {}


def _build_program():
    nc = bacc.Bacc("TRN2", target_bir_lowering=False, debug=False,
                   num_devices=N_CORES)

    def din(name, shape, dt=f32):
        return nc.dram_tensor(name, shape, dt, kind="ExternalInput").ap()

    xe_slab_d = din("xe_slab", [128, NSLAB], bf16)
    pred_s = din("pred_s", [NITER, 128, COLS], bf16)
    w2dt_d = din("w2dt", [128, 27 * 256], bf16)
    gmask_d = din("gmask", [128, 16])
    gexp_d = din("gexp", [16, 128])
    gnw_d = din("gnw", [128, 2])
    gapbT_d = din("gapbT", [128, 2])
    ones13_d = din("ones13", [1, 3], bf16)
    gapbr_d = din("gapbr", [1, 256], bf16)
    bcrow_d = din("bcrow", [1, 256], bf16)
    ba1row_d = din("ba1row", [1, 16], bf16)
    ba2row_d = din("ba2row", [1, 128], bf16)
    w_cfT_d = din("w_cfT", [128, 2 * 512], bf16)
    bcf3_d = din("bcf3", [3, 4 * 128], bf16)
    id3_d = din("id3", [3, 3], bf16)
    w_cT_d = din("w_cT", [128, 4 * 256], bf16)
    w_a1T_d = din("w_a1T", [128, 2 * 16], bf16)
    w_a2T4_d = din("w_a2T4", [16, 128], bf16)
    wseg12_d = din("wseg12", [128, 3])
    bseg76_d = din("bseg76", [76, 1])

    out_d = nc.dram_tensor("out_s", [12, NG], bf16,
                           kind="ExternalOutput").ap()

    with tile.TileContext(nc) as tc:
        with tc.tile_pool(name="small", bufs=1) as sp, \
             tc.tile_pool(name="pred", bufs=8) as pp, \
             tc.tile_pool(name="outp", bufs=2) as op, \
             tc.tile_pool(name="hps", bufs=2, space="PSUM") as hps, \
             tc.tile_pool(name="sps", bufs=6, space="PSUM") as sps:
          with tc.tile_pool(name="headbig", bufs=1) as hb, \
               tc.tile_pool(name="scratch", bufs=2) as scp:

            # ---- head-critical loads first (sync-queue FIFO priority) ----
            xe_slab = hb.tile([128, NSLAB], bf16)
            for i in range(NSC):
                nc.sync.dma_start(xe_slab[:, i * CSC:(i + 1) * CSC],
                                  xe_slab_d[:, i * CSC:(i + 1) * CSC])
            gmask = sp.tile([128, 16], f32)
            nc.sync.dma_start(gmask[:], gmask_d[:])
            gexp = sp.tile([16, 128], f32)
            nc.sync.dma_start(gexp[:], gexp_d[:])
            gnw = sp.tile([128, 2], f32)
            nc.sync.dma_start(gnw[:], gnw_d[:])
            w2dt = hb.tile([128, 27 * 256], bf16)
            nc.sync.dma_start(w2dt[:], w2dt_d[:])
            gapbT = sp.tile([128, 2], f32)
            nc.sync.dma_start(gapbT[:], gapbT_d[:])
            ones13 = sp.tile([1, 3], bf16)
            nc.sync.dma_start(ones13[:], ones13_d[:])
            gapbr = sp.tile([1, 256], bf16)
            nc.sync.dma_start(gapbr[:], gapbr_d[:])
            bcrow = sp.tile([1, 256], bf16)
            nc.sync.dma_start(bcrow[:], bcrow_d[:])
            ba1row = sp.tile([1, 16], bf16)
            nc.sync.dma_start(ba1row[:], ba1row_d[:])
            ba2row = sp.tile([1, 128], bf16)
            nc.sync.dma_start(ba2row[:], ba2row_d[:])
            w_cfT = hb.tile([128, 2 * 512], bf16)
            nc.sync.dma_start(w_cfT[:], w_cfT_d[:])
            bcf3 = sp.tile([3, 4 * 128], bf16)
            nc.sync.dma_start(bcf3[:], bcf3_d[:])
            id3 = sp.tile([3, 3], bf16)
            nc.sync.dma_start(id3[:], id3_d[:])
            w_cT = hb.tile([128, 4 * 256], bf16)
            nc.sync.dma_start(w_cT[:], w_cT_d[:])
            w_a1T = sp.tile([128, 2 * 16], bf16)
            nc.sync.dma_start(w_a1T[:], w_a1T_d[:])
            w_a2T4 = sp.tile([16, 128], bf16)
            nc.sync.dma_start(w_a2T4[:], w_a2T4_d[:])
            wseg12 = sp.tile([128, 3], f32)
            nc.sync.dma_start(wseg12[:], wseg12_d[:])
            bd = sp.tile([128, 12], bf16)
            nc.vector.memset(bd[:], 0)
            bseg76 = sp.tile([76, 1], f32)
            nc.sync.dma_start(bseg76[:], bseg76_d[:])

            # ---- pred stream loads ----
            pts = []
            for t in range(NITER):
                pt = pp.tile([128, COLS], bf16, tag="pt")
                nc.sync.dma_start(pt[:], pred_s[t])
                pts.append(pt)

            # ---- GN stats, chunked to overlap the slab DMA ----
            sqp = sp.tile([128, NSC], f32)
            smp = sp.tile([128, NSC], f32)
            for i in range(NSC):
                ch_ = xe_slab[:, i * CSC:(i + 1) * CSC]
                sq_sc = scp.tile([128, CSC], bf16, tag="sc", bufs=1)
                nc.scalar.activation(sq_sc[:], ch_, AF.Square,
                                     accum_out=sqp[:, i:i + 1])
                nc.vector.tensor_reduce(smp[:, i:i + 1], ch_, AX.X, ALU.add)
            stat2 = sp.tile([128, 2], f32)  # cols: sum, sumsq
            nc.vector.tensor_reduce(stat2[:, 0:1], smp[:], AX.X, ALU.add)
            nc.vector.tensor_reduce(stat2[:, 1:2], sqp[:], AX.X, ALU.add)

            # group-sum via mask matmul -> [16, 2]
            g2 = hps.tile([16, 2], f32, tag="hps")
            nc.tensor.matmul(g2[:], gmask[:], stat2[:], start=True, stop=True)
            gsum = sp.tile([16, 2], f32)
            nc.vector.tensor_copy(gsum[:], g2[:])

            # -mu, rsqrt(var+eps) per group -> mr [16, 2]
            mr = sp.tile([16, 2], f32)  # cols: -mu, rs
            nc.scalar.mul(mr[:, 0:1], gsum[:, 0:1], -1.0 / NGRP)
            # n*var = sumsq + sum*(-mu); sd = sqrt(n*var/n + eps)
            nvar = sp.tile([16, 1], f32)
            nc.vector.scalar_tensor_tensor(nvar[:], gsum[:, 0:1],
                                           mr[:, 0:1], gsum[:, 1:2],
                                           ALU.mult, ALU.add)
            epst = sp.tile([16, 1], f32)
            nc.vector.memset(epst[:], float(EPS))
            sd = sp.tile([16, 1], f32)
            nc.scalar.activation(sd[:], nvar[:], AF.Sqrt,
                                 bias=epst[:, 0:1], scale=1.0 / NGRP)
            nc.vector.reciprocal(mr[:, 1:2], sd[:])

            # expand groups -> channels: chmr [128, 2] = (-mu_c, rs_c)
            ch2 = hps.tile([128, 2], f32, tag="hps")
            nc.tensor.matmul(ch2[:], gexp[:], mr[:], start=True, stop=True)
            chmr = sp.tile([128, 2], f32)
            nc.vector.tensor_copy(chmr[:], ch2[:])
            # scale_c = rs*gamma ; bias_c = beta + (-mu)*scale
            # u = bias/scale = beta/scale + (-mu)
            scale = sp.tile([128, 1], f32)
            nc.vector.tensor_scalar_mul(scale[:], chmr[:, 1:2], gnw[:, 0:1])
            bias = sp.tile([128, 1], f32)
            nc.vector.scalar_tensor_tensor(bias[:], chmr[:, 0:1], scale[:, 0:1],
                                           gnw[:, 1:2], ALU.mult, ALU.add)
            rsc = sp.tile([128, 1], f32)
            nc.vector.reciprocal(rsc[:], scale[:])
            u = sp.tile([128, 1], f32)
            nc.vector.scalar_tensor_tensor(u[:], gnw[:, 1:2], rsc[:, 0:1],
                                           chmr[:, 0:1], ALU.mult, ALU.add)

            # ---- R' = max(x+u, 0) in ONE DVE 4x op (assumes gamma>0) ----
            Rp = hb.tile([128, NSLAB], bf16)
            nc.vector.tensor_scalar(Rp[:], xe_slab[:], u[:, 0:1], 0.0,
                                    ALU.add, ALU.max)

            # ---- 27 window sums + interleaved x_feat accumulation ----
            S = sp.tile([128, 27], f32)
            Sb = sp.tile([128, 27], bf16)
            wsl = xe_slab[:].rearrange("p (d w h) -> p d w h",
                                       d=24, w=24, h=24)
            wrp = Rp[:].rearrange("p (d w h) -> p d w h", d=24, w=24, h=24)
            xfr = hps.tile([1, 256], f32, tag="hps")

            def xf_mm(o, first, last):
                nc.tensor.matmul(xfr[:], Sb[:, o:o + 1],
                                 w2dt[:, o * 256:(o + 1) * 256],
                                 start=first, stop=last)

            # DVE windows first (from R'), then ACT windows (inline norm)
            for o in range(NWACT, 27):
                od, ow, oh = o // 9, (o // 3) % 3, o % 3
                win = wrp[:, od:od + 21:2, ow:ow + 21:2, oh:oh + 21:2]
                nc.vector.tensor_reduce(S[:, o:o + 1], win, AX.XYZ, ALU.add)
                nc.vector.tensor_scalar(Sb[:, o:o + 1], S[:, o:o + 1],
                                        scale[:, 0:1], None, ALU.mult)
                xf_mm(o, o == NWACT, False)
            for o in range(NWACT):
                od, ow, oh = o // 9, (o // 3) % 3, o % 3
                win = wsl[:, od:od + 21:2, ow:ow + 21:2, oh:oh + 21:2]
                wscr = scp.tile([128, 1331], bf16, tag="wsc", bufs=1)
                nc.scalar.activation(wscr[:], win, AF.Relu,
                                     bias=bias[:, 0:1], scale=scale[:, 0:1],
                                     accum_out=S[:, o:o + 1])
                nc.vector.tensor_copy(Sb[:, o:o + 1], S[:, o:o + 1])
                xf_mm(o, False, o == NWACT - 1)

            xfsb = sp.tile([1, 256], bf16)
            nc.scalar.activation(xfsb[:], xfr[:], AF.Copy)
            # xcT chunks via broadcast matmuls: (x_feat + gap_b) x ones[1,3]
            xcT = sp.tile([128, 6], bf16)
            for pc in range(2):
                xcp = hps.tile([128, 3], f32, tag="hps")
                nc.tensor.matmul(xcp[:], xfsb[0:1, pc * 128:(pc + 1) * 128],
                                 ones13[:], start=True, stop=False)
                nc.tensor.matmul(xcp[:], gapbr[0:1, pc * 128:(pc + 1) * 128],
                                 ones13[:], start=False, stop=True)
                nc.vector.tensor_copy(xcT[:, pc * 3:pc * 3 + 3], xcp[:])

            # ---- MLP1: p3T = relu(Wx @ x_feat + (We@emb + b_cf)).T ----
            p3T = sp.tile([128, 4 * 3], bf16)
            p1 = hps.tile([128, 12], f32, tag="hps")
            for oc in range(4):
                for pc in range(2):
                    nc.tensor.matmul(
                        p1[:, oc * 3:oc * 3 + 3],
                        w_cfT[:, pc * 512 + oc * 128: pc * 512 + oc * 128 + 128],
                        xcT[:, pc * 3:pc * 3 + 3],
                        start=(pc == 0), stop=False)
                nc.tensor.matmul(p1[:, oc * 3:oc * 3 + 3],
                                 bcf3[:, oc * 128:(oc + 1) * 128],
                                 id3[:], start=False, stop=True)
            nc.scalar.activation(p3T[:], p1[:], AF.Relu)

            # ---- MLP2: c3T [128, 2*3] ----
            c3T = sp.tile([128, 2 * 3], bf16)
            c1 = hps.tile([128, 6], f32, tag="hps")
            for oc in range(2):
                for pc in range(4):
                    nc.tensor.matmul(
                        c1[:, oc * 3:oc * 3 + 3],
                        w_cT[:, pc * 256 + oc * 128: pc * 256 + oc * 128 + 128],
                        p3T[:, pc * 3:pc * 3 + 3],
                        start=(pc == 0), stop=False)
                nc.tensor.matmul(c1[:, oc * 3:oc * 3 + 3],
                                 bcrow[0:1, oc * 128:(oc + 1) * 128],
                                 ones13[:], start=False, stop=True)
            nc.scalar.activation(c3T[:], c1[:], AF.Identity)

            # ---- MLP3: hT [16, 3] ----
            h1 = hps.tile([16, 3], f32, tag="hps")
            for pc in range(2):
                nc.tensor.matmul(h1[:], w_a1T[:, pc * 16:pc * 16 + 16],
                                 c3T[:, pc * 3:pc * 3 + 3],
                                 start=(pc == 0), stop=False)
            nc.tensor.matmul(h1[:], ba1row[:], ones13[:],
                             start=False, stop=True)
            hT = sp.tile([16, 3], bf16)
            nc.scalar.activation(hT[:], h1[:], AF.Relu)

            # ---- MLP4, 4x-replicated: gT [128, 3] = sigmoid(...) ----
            g1 = hps.tile([128, 3], f32, tag="hps")
            nc.tensor.matmul(g1[:], w_a2T4[:], hT[:], start=True, stop=False)
            nc.tensor.matmul(g1[:], ba2row[:], ones13[:],
                             start=False, stop=True)
            gT = sp.tile([128, 3], f32)
            nc.scalar.activation(gT[:], g1[:], AF.Sigmoid)

            # ---- effw -> block-diagonal bf16 lhsT [128, 12], no DMA ----
            for g in range(G):
                nc.vector.tensor_mul(bd[32 * g:32 * g + 32, 3 * g:3 * g + 3],
                                     gT[32 * g:32 * g + 32, :],
                                     wseg12[32 * g:32 * g + 32, :])

          # ---- main stream: 3-stacked matmuls, wide drains ----
          # out_s col layout: ((t*3 + s)*NBANK + b)*512 + i  (host reorders)
          for t in range(NITER):
            so = op.tile([76, NBANK * 512], bf16, tag="so")
            for b in range(NBANK):
              po = sps.tile([128, 512], f32, tag="po")
              for s in range(3):
                j = b * 3 + s
                nc.tensor.matmul(po[32 * s:32 * s + 12, :], bd[:],
                                 pts[t][:, j * 512:(j + 1) * 512],
                                 start=True, stop=True)
              dst = so[:, b * 512:(b + 1) * 512]
              if b % 2 == 0:
                  nc.scalar.activation(dst, po[0:76, :], AF.Identity,
                                       bias=bseg76[:, 0:1])
              else:
                  nc.vector.tensor_scalar_add(dst, po[0:76, :],
                                              bseg76[:, 0:1])
            for s in range(3):
              c0 = (t * 3 + s) * NBANK * 512
              eng = (nc.gpsimd, nc.scalar, nc.sync)[s]
              eng.dma_start(out_d[:, c0:c0 + NBANK * 512],
                            so[32 * s:32 * s + 12, :])

    nc.compile()
    return nc


def _prep_shared(inp):
    """Host-side weight transposes shared by all cores."""
    gn_g = np.asarray(inp["gn_g"], np.float32)
    gn_b = np.asarray(inp["gn_b"], np.float32)
    gap_b = np.asarray(inp["gap_b"], np.float32)
    gap_w = np.asarray(inp["gap_w"], np.float32)
    w_cf = np.asarray(inp["w_cf"], np.float32)
    b_cf = np.asarray(inp["b_cf"], np.float32)
    w_c = np.asarray(inp["w_c"], np.float32)
    b_c = np.asarray(inp["b_c"], np.float32)
    w_a1 = np.asarray(inp["w_a1"], np.float32)
    b_a1 = np.asarray(inp["b_a1"], np.float32)
    w_a2 = np.asarray(inp["w_a2"], np.float32)
    b_a2 = np.asarray(inp["b_a2"], np.float32)
    emb = np.asarray(inp["emb"], np.float32)
    w_seg = np.asarray(inp["w_seg"], np.float32)
    b_seg = np.asarray(inp["b_seg"], np.float32)

    assert np.all(gn_g > 0), "kernel assumes positive GroupNorm gamma"

    p = np.arange(128)
    gmask = (p[:, None] // 8 == np.arange(16)[None, :]).astype(np.float32)
    gexp = np.ascontiguousarray(gmask.T)

    # W2 transposed + mean-fold: w2dt[c, o*256+oc] = gap_w[oc, c, o]/1331
    w2 = gap_w.reshape(256, 128, 27)
    w2dt = np.ascontiguousarray(
        (w2.transpose(1, 2, 0) / np.float32(1331.0)).reshape(128, 27 * 256)
    ).astype(BF)

    # x-half of w_cf, transposed: [128, 2*512]
    wx = w_cf[:, 0:256].T                            # [256, 512]
    w_cfT = np.concatenate(
        [wx[128 * pc:128 * (pc + 1), :] for pc in range(2)],
        axis=1).astype(BF)
    # constant-folded emb-half + bias: bcf3[k, o] = b_cf[o] + We @ emb_k
    bcf3 = np.ascontiguousarray(
        b_cf[None, :] + emb @ w_cf[:, 256:512].T).astype(BF)    # [3, 512]
    id3 = np.eye(3, dtype=np.float32).astype(BF)
    w_cT = np.concatenate(
        [w_c.T[128 * pc:128 * (pc + 1), :] for pc in range(4)],
        axis=1).astype(BF)
    bcT = np.ascontiguousarray(b_c.reshape(2, 128).T)
    w_a1T = np.concatenate(
        [w_a1.T[128 * pc:128 * (pc + 1), :] for pc in range(2)],
        axis=1).astype(BF)
    ba1 = b_a1.reshape(16, 1)
    w_a2T4 = np.ascontiguousarray(np.tile(w_a2.T, (1, 4))).astype(BF)
    ba2r = np.tile(b_a2, 4).reshape(128, 1)
    wseg12 = np.ascontiguousarray(np.tile(w_seg.T, (4, 1)))  # [128, 3]

    gapbT = np.ascontiguousarray(gap_b.reshape(2, 128).T)    # [128, 2]
    gnw = np.ascontiguousarray(np.stack([gn_g, gn_b], axis=1))  # [128, 2]
    ones13 = np.ones((1, 3), np.float32).astype(BF)
    gapbr = gap_b.reshape(1, 256).astype(BF)
    bcrow = b_c.reshape(1, 256).astype(BF)
    ba1row = b_a1.reshape(1, 16).astype(BF)
    ba2row = np.tile(b_a2, 4).reshape(1, 128).astype(BF)
    bseg76 = np.zeros((76, 1), np.float32)
    for s in range(3):
        bseg76[32 * s:32 * s + 12, 0] = np.tile(b_seg, 4)[:12]

    return dict(gmask=gmask, gexp=gexp, gnw=gnw, w2dt=w2dt, gapbT=gapbT,
                ones13=ones13, gapbr=gapbr, bcrow=bcrow, ba1row=ba1row,
                ba2row=ba2row, w_cfT=w_cfT, bcf3=bcf3, id3=id3, w_cT=w_cT,
                w_a1T=w_a1T, w_a2T4=w_a2T4,
                wseg12=wseg12, bseg76=bseg76)


def kernel(**inputs):
    global LAST_EXEC_NS
    x_e = np.asarray(inputs["x_e"], np.float32)
    pred = np.asarray(inputs["pred"], np.float32)

    shared = _prep_shared(inputs)
    shared = {k: (np.ascontiguousarray(v) if v.dtype == BF
                  else np.ascontiguousarray(v, dtype=np.float32))
              for k, v in shared.items()}

    xe_bf = x_e.reshape(B, 128, NSLAB).astype(BF)        # [2, 128, 13824]
    pred_bf = pred.astype(BF)                            # [2, 32, 96, 96, 96]

    in_maps = []
    for r in range(N_CORES):
        b, dq = divmod(r, 4)
        m = dict(shared)

        ps = pred_bf[b, :, dq * 24:(dq + 1) * 24]        # [32, 24, 96, 96]
        ps = ps.reshape(32, G, NITER, COLS).transpose(2, 1, 0, 3)
        m["pred_s"] = np.ascontiguousarray(ps.reshape(NITER, 128, COLS))
        m["xe_slab"] = np.ascontiguousarray(xe_bf[b])
        in_maps.append(m)

    if "nc" not in _CACHE:
        _CACHE["nc"] = _build_program()
    nc = _CACHE["nc"]

    res = run_bass_kernel_spmd(nc, in_maps, list(range(N_CORES)),
                               trace=TRACE)
    LAST_EXEC_NS = res.exec_time_ns

    out = np.empty((B, K, 96, 96, 96), np.float32)
    for r in range(N_CORES):
        b, dq = divmod(r, 4)
        o = np.asarray(res.results[r]["out_s"]).astype(np.float32)  # [12, NG]
        # stored col order ((t*3+s)*NBANK+b)*512+i -> logical chunk t*NCH+b*3+s
        o = o.reshape(12, NITER, 3, NBANK, 512).transpose(0, 1, 3, 2, 4)
        o = o.reshape(12, NG)
        o = o.reshape(G, K, NG).transpose(1, 0, 2).reshape(K, NPOS)
        out[b, :, dq * 24:(dq + 1) * 24] = o.reshape(K, 24, 96, 96)
    return out
